# revision 1
# baseline (speedup 1.0000x reference)
"""Trainium2 Bass kernel for nn_AttentionBlock_78400333021395.

AttentionBlock: x -> x + ls1*MHSA(BN(x)) -> + ls2*ConvFFN(BN(.))
  B=64, C=512, H=W=14 (N=196 tokens), 16 heads x 32 dim, FFN hidden 2048,
  depthwise 7x7 conv, inference BN, layer-scale 1e-5.

Sharding: data-parallel over batch, 8 batches per core on 8 NeuronCores.
All BN affines / layer scales / qk scale are folded into matmul weights and
biases on the host. Matmuls run in bf16 (residual path stays f32; all
approximation error is scaled by layer_scale=1e-5 so precision headroom is
large). The depthwise conv is split across PE (block-diag matmuls), DVE
(scalar_tensor_tensor FMA taps) and GPSIMD.
"""

import numpy as np
import ml_dtypes

import concourse.bass as bass
import concourse.tile as tile
from concourse import bacc, mybir
from concourse.bass_utils import run_bass_kernel_spmd

# ---- problem constants (hardcoded per harness contract) ----
B, C, H, W = 64, 512, 14, 14
N = H * W  # 196
NH, D = 16, 32
HID = 2048
EPS = 1e-5
NCORES = 8
BPC = B // NCORES  # 8 batches per core
TOK = BPC * N      # 1568 tokens per core
P = 128
CCH = C // P       # 4 channel chunks
HCH = HID // P     # 16 hidden chunks
MCHUNKS = [(0, 128), (128, 68)]      # token chunks within a batch (196 = 128+68)
PADW = 20                            # padded spatial width (14 + 2*3)

# depthwise tap assignment: (dy, dx) pairs
PE_TAPS = ([(dy, dx) for dy in range(7) for dx in (1, 3, 5)]
           + [(dy, 0) for dy in range(7)]
           + [(dy, 2) for dy in range(3)])                          # 31 taps
GP_TAPS = []  # stt unsupported on Pool engine (NCC_IXCG966)
DVE_TAPS = [(dy, dx) for dy in range(7) for dx in (0, 2, 4, 6)
            if not (dx == 0 or (dx == 2 and dy < 3))]               # 18 taps
assert len(PE_TAPS) + len(GP_TAPS) + len(DVE_TAPS) == 49
assert set(PE_TAPS + GP_TAPS + DVE_TAPS) == {(dy, dx) for dy in range(7) for dx in range(7)}

F32 = mybir.dt.float32
BF16 = mybir.dt.bfloat16
AF = mybir.ActivationFunctionType
ALU = mybir.AluOpType

_CACHE = {}
DEBUG = False
ATTN_ONLY = False
ATTN_BATCHES = 8
REPS = 1


def _build_nc():
    nc = bacc.Bacc("TRN2", target_bir_lowering=False, debug=False,
                   num_devices=NCORES)

    xin = nc.dram_tensor("xin", [CCH, P, BPC, N], F32, kind="ExternalInput")
    qkvw = nc.dram_tensor("qkvw", [CCH, P, 3 * C], BF16, kind="ExternalInput")
    qb = nc.dram_tensor("qb", [P, CCH], F32, kind="ExternalInput")
    projw = nc.dram_tensor("projw", [CCH, P, C], BF16, kind="ExternalInput")
    projb = nc.dram_tensor("projb", [P, CCH], F32, kind="ExternalInput")
    fc1w = nc.dram_tensor("fc1w", [CCH, P, HID], BF16, kind="ExternalInput")
    fc1b = nc.dram_tensor("fc1b", [P, HCH], F32, kind="ExternalInput")
    fc2w = nc.dram_tensor("fc2w", [HCH, P, C], BF16, kind="ExternalInput")
    fc2b = nc.dram_tensor("fc2b", [P, CCH], F32, kind="ExternalInput")
    s1d = nc.dram_tensor("s1d", [P, CCH], F32, kind="ExternalInput")
    t1d = nc.dram_tensor("t1d", [P, CCH], F32, kind="ExternalInput")
    dwtap = nc.dram_tensor("dwtap", [P, CCH, 49], F32, kind="ExternalInput")
    dwdiag = nc.dram_tensor("dwdiag", [len(PE_TAPS), CCH, P, P], BF16,
                            kind="ExternalInput")
    selw = nc.dram_tensor("selw", [2, P], F32, kind="ExternalInput")
    out = nc.dram_tensor("out", [CCH, P, BPC, N], F32, kind="ExternalOutput")
    dbg = {}
    if DEBUG:
        dbg["q0"] = nc.dram_tensor("dbg_q0", [P, N], F32, kind="ExternalOutput")
        dbg["k0"] = nc.dram_tensor("dbg_k0", [P, N], F32, kind="ExternalOutput")
        dbg["pt0"] = nc.dram_tensor("dbg_pt0", [P, 4, 2 * N], F32, kind="ExternalOutput")
        dbg["av0"] = nc.dram_tensor("dbg_av0", [P, N], F32, kind="ExternalOutput")
        dbg["rec0"] = nc.dram_tensor("dbg_rec0", [2, N], F32, kind="ExternalOutput")
        dbg["ot0"] = nc.dram_tensor("dbg_ot0", [P, N], F32, kind="ExternalOutput")

    with tile.TileContext(nc) as tc:
        for _rep in range(REPS):
            _emit(nc, tc, xin, qkvw, qb, projw, projb, fc1w, fc1b, fc2w,
                  fc2b, s1d, t1d, dwtap, dwdiag, selw, out, dbg)
    nc.compile()
    return nc


def _emit(nc, tc, xin, qkvw, qb, projw, projb, fc1w, fc1b, fc2w, fc2b,
          s1d, t1d, dwtap, dwdiag, selw, out, dbg={}):
    from contextlib import ExitStack
    ctx = ExitStack()
    wp = ctx.enter_context(tc.tile_pool(name="wp", bufs=1))
    xst = ctx.enter_context(tc.tile_pool(name="xst", bufs=1))
    xw = ctx.enter_context(tc.tile_pool(name="xw", bufs=1))
    x2p = ctx.enter_context(tc.tile_pool(name="x2p", bufs=1))
    padp = ctx.enter_context(tc.tile_pool(name="padp", bufs=5))
    accp = ctx.enter_context(tc.tile_pool(name="accp", bufs=6))
    qkp = ctx.enter_context(tc.tile_pool(name="qkp", bufs=6))
    vp = ctx.enter_context(tc.tile_pool(name="vp", bufs=4))
    ptp = ctx.enter_context(tc.tile_pool(name="ptp", bufs=9))
    otp = ctx.enter_context(tc.tile_pool(name="otp", bufs=8))
    rcp = ctx.enter_context(tc.tile_pool(name="rcp", bufs=3))
    oup = ctx.enter_context(tc.tile_pool(name="oup", bufs=12))
    dtp = ctx.enter_context(tc.tile_pool(name="dtp", bufs=2))
    gpp = ctx.enter_context(tc.tile_pool(name="gpp", bufs=2))
    xrp = ctx.enter_context(tc.tile_pool(name="xrp", bufs=4))
    gep = ctx.enter_context(tc.tile_pool(name="gep", bufs=2))
    outp = ctx.enter_context(tc.tile_pool(name="outp", bufs=4))
    dgp = ctx.enter_context(tc.tile_pool(name="dgp", bufs=8))
    psA = ctx.enter_context(tc.tile_pool(name="psA", bufs=4, space="PSUM"))
    ps2 = ctx.enter_context(tc.tile_pool(name="ps2", bufs=2, space="PSUM"))

    # ---- x first (gates the first qkv matmul), then attention weights,
    # then FFN weights (not needed until ~half-way through the kernel) ----
    xbf = []
    for cc in range(CCH):
        xs = xst.tile([P, TOK], F32)
        nc.sync.dma_start(xs[:], xin[cc].rearrange("p b n -> p (b n)"))
        xb = xw.tile([P, TOK], BF16, tag=f"xbf{cc}")
        nc.vector.tensor_copy(xb[:], xs[:])
        xbf.append(xb)

    def ldvec(dram, shape, tag):
        t = wp.tile(shape, F32, tag=tag, name=tag)
        nc.sync.dma_start(t[:], dram[:])
        return t

    qkv_sb, pw_sb = [], []
    for kt in range(CCH):
        t = wp.tile([P, 3 * C], BF16, tag=f"qkvw{kt}", name=f"qkvw{kt}")
        nc.sync.dma_start(t[:], qkvw[kt])
        qkv_sb.append(t)
    qb_sb = ldvec(qb, [P, CCH], "qb")
    sel_sb = wp.tile([2, P], F32, tag="sel")
    nc.sync.dma_start(sel_sb[:], selw[:])
    for kt in range(CCH):
        t = wp.tile([P, C], BF16, tag=f"projw{kt}", name=f"projw{kt}")
        nc.sync.dma_start(t[:], projw[kt])
        pw_sb.append(t)
    pb_sb = ldvec(projb, [P, CCH], "pb")
    s1_sb = ldvec(s1d, [P, CCH], "s1")
    t1_sb = ldvec(t1d, [P, CCH], "t1")
    dwt_sb = wp.tile([P, CCH, 49], F32, tag="dwt")
    nc.sync.dma_start(dwt_sb[:], dwtap[:])
    f1_sb, f2_sb = [], []
    for kt in range(CCH):
        t = wp.tile([P, HID], BF16, tag=f"fc1w{kt}", name=f"fc1w{kt}")
        nc.sync.dma_start(t[:], fc1w[kt])
        f1_sb.append(t)
    f1b_sb = ldvec(fc1b, [P, HCH], "f1b")
    for kt in range(HCH):
        t = wp.tile([P, C], BF16, tag=f"fc2w{kt}", name=f"fc2w{kt}")
        nc.sync.dma_start(t[:], fc2w[kt])
        f2_sb.append(t)
    f2b_sb = ldvec(fc2b, [P, CCH], "f2b")

    x2 = [x2p.tile([P, TOK], F32, tag=f"x2_{cc}", name=f"x2_{cc}") for cc in range(CCH)]

    # ================= attention (per batch) =================
    def attn_batch(b):
        xrs = []
        for cc in range(CCH):
            xr = xrp.tile([P, N], F32, tag="xr", name=f"xr{b}_{cc}")
            nc.sync.dma_start(xr[:], xin[cc, :, b, :])
            xrs.append(xr)
        # V blocks of 66 cols per head. Even head h: [v(32) zeros(32) 1 0],
        # odd head: [zeros(32) v(32) 0 1] -> A@V psum rows: O_even 0:32,
        # O_odd 32:64, denom_even row 64, denom_odd row 65.
        V = []
        for (moff, mlen) in MCHUNKS:
            psv = psA.tile([P, 512], F32, tag="ps")
            for kt in range(CCH):
                nc.tensor.matmul(psv[:mlen, :],
                                 xbf[kt][:, b * N + moff: b * N + moff + mlen],
                                 qkv_sb[kt][:, 2 * C:3 * C],
                                 start=(kt == 0), stop=(kt == CCH - 1))
            vt = vp.tile([P, NH * 66], BF16, tag="v")
            nc.gpsimd.memset(vt[:], 0.0)
            nc.gpsimd.memset(vt[:, 64:64 + 132 * 7 + 1:132], 1.0)
            nc.gpsimd.memset(vt[:, 131:131 + 132 * 7 + 1:132], 1.0)
            pv = psv[:mlen].rearrange("p (h d) -> p h d", d=32)
            vv = vt[:mlen].rearrange("p (q e) -> p q e", e=132)
            nc.vector.tensor_copy(vv[:, :, 0:32], pv[:, 0:NH:2, :])
            nc.vector.tensor_copy(vv[:, :, 98:130], pv[:, 1:NH:2, :])
            V.append(vt)

        Q, K = [], []
        for g in range(CCH):
            psqk = psA.tile([P, 512], F32, tag="ps")
            for kt in range(CCH):
                nc.tensor.matmul(psqk[:, :N], qkv_sb[kt][:, g * P:(g + 1) * P],
                                 xbf[kt][:, b * N:(b + 1) * N],
                                 start=(kt == 0), stop=(kt == CCH - 1))
            for kt in range(CCH):
                nc.tensor.matmul(psqk[:, N:2 * N],
                                 qkv_sb[kt][:, C + g * P:C + (g + 1) * P],
                                 xbf[kt][:, b * N:(b + 1) * N],
                                 start=(kt == 0), stop=(kt == CCH - 1))
            qt = qkp.tile([P, N], BF16, tag="q")
            nc.vector.tensor_scalar_add(qt[:], psqk[:, :N], qb_sb[:, g:g + 1])
            Q.append(qt)
            kt_ = qkp.tile([P, N], BF16, tag="k")
            nc.vector.tensor_copy(kt_[:], psqk[:, N:2 * N])
            K.append(kt_)
            if dbg and b == 0 and g == 0:
                nc.gpsimd.dma_start(dbg["q0"][:], qt[:])
                nc.gpsimd.dma_start(dbg["k0"][:], kt_[:])

        # S^T = K^T Q per head (row-tiled), 2 heads per 2-bank psum tile
        PT = {}
        for g in range(CCH):
            for jh in range(2):
                pss = ps2.tile([P, 2, 512], F32, tag="ps2")
                for j2 in range(2):
                    j = 2 * jh + j2
                    for mc, (moff, mlen) in enumerate(MCHUNKS):
                        nc.tensor.matmul(
                            pss[:mlen, j2, mc * N:mc * N + N],
                            K[g][32 * j:32 * j + 32, moff:moff + mlen],
                            Q[g][32 * j:32 * j + 32, :],
                            start=True, stop=True, tile_position=(32 * j, 0))
                ptt = ptp.tile([P, 2, 2 * N], BF16, tag="pt",
                               name=f"pt{b}_{g}_{jh}")
                nc.scalar.activation(ptt[:, :, 0:N], pss[:, :, 0:N], AF.Exp)
                nc.scalar.activation(ptt[:68, :, N:2 * N],
                                     pss[:68, :, N:2 * N], AF.Exp)
                PT[(g, jh)] = ptt
                if dbg and b == 0 and g == 0:
                    nc.gpsimd.dma_start(
                        dbg["pt0"][:, 2 * jh:2 * jh + 2, :], ptt[:])

        # A@V -> [O_even(0:32); O_odd(32:64); denom_e(64); denom_o(65)]
        OT = [otp.tile([P, N], BF16, tag="ot", name=f"ot_{b}_{_}") for _ in range(CCH)]
        for pr in range(NH // 2):
            psav = psA.tile([P, 512], F32, tag="ps")
            nmm = 0
            for mc, (moff, mlen) in enumerate(MCHUNKS):
                for e in range(2):
                    h = 2 * pr + e
                    g, j = divmod(h, 4)
                    nc.tensor.matmul(psav[:66, :N],
                                     V[mc][:mlen, h * 66:h * 66 + 66],
                                     PT[(g, j // 2)][:mlen, j % 2,
                                                     mc * N:mc * N + N],
                                     start=(nmm == 0), stop=(nmm == 3))
                    nmm += 1
            if dbg and b == 0 and pr == 0:
                dav_ = xst.tile([P, N], F32, tag="dbgav", name="dbgav")
                nc.gpsimd.memset(dav_[:], 0.0)
                nc.vector.tensor_copy(dav_[:66, :], psav[:66, :N])
                nc.sync.dma_start(dbg["av0"][:], dav_[:])
            oub = oup.tile([64, N], BF16, tag="oub", name=f"oub_{b}_{pr}")
            nc.scalar.activation(oub[:], psav[:64, :N], AF.Copy)
            den2 = rcp.tile([2, N], F32, tag="den", name=f"den_{b}_{pr}")
            nc.scalar.activation(den2[:], psav[64:66, :N], AF.Copy)
            rec2 = rcp.tile([2, N], F32, tag="rec", name=f"rec_{b}_{pr}")
            nc.vector.reciprocal_approx_fast(rec2[:], den2[:])
            if dbg and b == 0 and pr == 0:
                nc.sync.dma_start(dbg["rec0"][:], rec2[:])
            psrb = psA.tile([P, 512], F32, tag="ps")
            nc.tensor.matmul(psrb[:64, :N], sel_sb[:, 0:64], rec2[:],
                             start=True, stop=True)
            g = pr // 2
            roff = 64 * (pr % 2)
            nc.vector.tensor_mul(OT[g][roff:roff + 64, :],
                                 oub[:], psrb[:64, :N])

        if dbg and b == 0:
            nc.gpsimd.dma_start(dbg["ot0"][:], OT[0][:])

        # proj + layer-scaled residual: x2 = x + (proj_psum + pb)
        for ch in range(CCH // 2):
            psp = psA.tile([P, 512], F32, tag="ps")
            for e in range(2):
                cc = 2 * ch + e
                for g in range(CCH):
                    nc.tensor.matmul(psp[:, e * N:e * N + N],
                                     pw_sb[g][:, cc * P:(cc + 1) * P],
                                     OT[g][:], start=(g == 0),
                                     stop=(g == CCH - 1))
            for e in range(2):
                cc = 2 * ch + e
                nc.vector.scalar_tensor_tensor(
                    x2[cc][:, b * N:(b + 1) * N], psp[:, e * N:e * N + N],
                    pb_sb[:, cc:cc + 1], xrs[cc][:], ALU.add, ALU.add)

    # ================= ConvFFN (per half of 4 batches) =================
    HB = BPC // 2                        # 4 batches per half
    NR2 = HB * PADW                      # 80 merged rows
    NPAD2 = HB * PADW * PADW + 6 * PADW

    def ffn_half(hf):
        b0 = hf * HB
        daccs = []
        for cc in range(CCH):
            xp = padp.tile([P, NPAD2], BF16, tag="xpad",
                           name=f"xpad{hf}_{cc}")
            nc.gpsimd.memset(xp[:], 0.0)
            xpv = xp[:, :HB * PADW * PADW].rearrange(
                "p (b y x) -> p b y x", b=HB, y=PADW)
            nc.vector.tensor_scalar(
                xpv[:, :, 3:3 + H, 3:3 + W],
                x2[cc][:, b0 * N:(b0 + HB) * N].rearrange(
                    "p (b y x) -> p b y x", b=HB, y=H),
                s1_sb[:, cc:cc + 1], t1_sb[:, cc:cc + 1], ALU.mult, ALU.add)
            xpr = xp[:, :(NR2 + 6) * PADW].rearrange("p (r c) -> p r c",
                                                     c=PADW)
            # PE taps: block-diag matmuls, one short-tenure psA tile per
            # 2-batch slice
            psds = [psA.tile([P, 512], F32, tag="ps",
                             name=f"pst{hf}_{cc}_{si}") for si in range(2)]
            for i, (dy, dx) in enumerate(PE_TAPS):
                dg = dgp.tile([P, P], BF16, tag="dg")
                nc.sync.dma_start(dg[:], dwdiag[i, cc])
                for si in range(2):
                    nc.tensor.matmul(
                        psds[si][:, :392], dg[:],
                        xpv[:, 2 * si:2 * si + 2, dy:dy + H, dx:dx + W],
                        start=(i == 0), stop=(i == len(PE_TAPS) - 1))

            # DVE taps: tensor_scalar (4x mode) into tmp, then 2x-mode
            # tensor_tensor accumulate (stt would run at 1x: no perf uops)
            da = accp.tile([P, NR2 * W], BF16, tag="dacc",
                           name=f"dacc{hf}_{cc}")
            dav = da.rearrange("p (r x) -> p r x", x=W)
            for i, (dy, dx) in enumerate(DVE_TAPS):
                ti = cc * 49 + dy * 7 + dx
                sc = dwt_sb.rearrange("p c t -> p (c t)")[:, ti:ti + 1]
                src_ = xpr[:, dy:dy + NR2, dx:dx + W]
                if i == 0:
                    nc.vector.tensor_scalar(dav[:], src_, sc, None, ALU.mult)
                else:
                    tmp = dtp.tile([P, NR2 * W], BF16, tag="dtmp",
                                    name=f"dtmp{hf}_{cc}_{i}")
                    nc.vector.tensor_scalar(
                        tmp.rearrange("p (r x) -> p r x", x=W)[:], src_, sc,
                        None, ALU.mult)
                    nc.vector.tensor_tensor(da[:], da[:], tmp[:], ALU.add)
            # GPSIMD taps
            if GP_TAPS:
                ga = gpp.tile([P, NR2 * W], BF16, tag="gacc",
                               name=f"gacc{hf}_{cc}")
                gav = ga.rearrange("p (r x) -> p r x", x=W)
                for i, (dy, dx) in enumerate(GP_TAPS):
                    ti = cc * 49 + dy * 7 + dx
                    sc = dwt_sb.rearrange("p c t -> p (c t)")[:, ti:ti + 1]
                    src_ = xpr[:, dy:dy + NR2, dx:dx + W]
                    if i == 0:
                        nc.gpsimd.tensor_scalar(gav[:], src_, sc, None,
                                                ALU.mult)
                    else:
                        nc.gpsimd.scalar_tensor_tensor(gav[:], src_, sc,
                                                       gav[:], ALU.mult,
                                                       ALU.add)
                nc.vector.tensor_tensor(da[:], da[:], ga[:], ALU.add)
            # combine PE psum taps
            dv4 = da.rearrange("p (b y x) -> p b y x", b=HB, y=PADW)
            for bl in range(HB):
                nc.vector.tensor_tensor(
                    dv4[:, bl, 0:H, :], dv4[:, bl, 0:H, :],
                    psds[bl // 2][:, (bl % 2) * N:(bl % 2) * N + N]
                    .rearrange("p (y x) -> p y x", x=W), ALU.add)
            daccs.append(da)

        # fc1 -> gelu -> fc2 -> residual out, per token slice of 392
        for si in range(2):
            s = 2 * hf + si
            ge = gep.tile([P, HCH, 392], BF16, tag="ge", name=f"ge{s}")
            for hc in range(HCH):
                psf = psA.tile([P, 512], F32, tag="ps")
                for kt in range(CCH):
                    nc.tensor.matmul(
                        psf[:, :392], f1_sb[kt][:, hc * P:(hc + 1) * P],
                        daccs[kt].rearrange("p (b y x) -> p b y x", b=HB,
                                            y=PADW)[:, 2 * si:2 * si + 2,
                                                    0:H, :],
                        start=(kt == 0), stop=(kt == CCH - 1))
                nc.scalar.activation(ge[:, hc, :], psf[:, :392], AF.Gelu,
                                     bias=f1b_sb[:, hc:hc + 1])
            for cc in range(CCH):
                psf2 = psA.tile([P, 512], F32, tag="ps")
                for kt in range(HCH):
                    nc.tensor.matmul(psf2[:, :392],
                                     f2_sb[kt][:, cc * P:(cc + 1) * P],
                                     ge[:, kt, :], start=(kt == 0),
                                     stop=(kt == HCH - 1))
                ot = outp.tile([P, 392], F32, tag="oo")
                nc.vector.scalar_tensor_tensor(
                    ot[:], psf2[:, :392], f2b_sb[:, cc:cc + 1],
                    x2[cc][:, s * 392:(s + 1) * 392], ALU.add, ALU.add)
                nc.sync.dma_start(out[cc, :, 2 * s:2 * s + 2, :],
                                  ot.rearrange("p (b n) -> p b n", b=2))

    nb = ATTN_BATCHES
    for b in range(min(4, nb)):
        attn_batch(b)
    if not ATTN_ONLY:
        ffn_half(0)
    for b in range(4, nb):
        attn_batch(b)
    if not ATTN_ONLY:
        ffn_half(1)

    ctx.close()


def _prep_inputs(x, bn_g, bn_b, bn_m, bn_v, qkv_w, proj_w, proj_b,
                 dw_w, fbn_g, fbn_b, fbn_m, fbn_v, fc1_w, fc1_b, fc2_w, fc2_b,
                 ls1, ls2):
    """Host-side folding of BN/layer-scale into weights; returns per-core in_maps."""
    f32 = np.float32
    bf = ml_dtypes.bfloat16
    x = np.asarray(x, f32)
    ls1v = np.asarray(ls1, f32).reshape(C)
    ls2v = np.asarray(ls2, f32).reshape(C)

    s1 = np.asarray(bn_g, f32) / np.sqrt(np.asarray(bn_v, f32) + EPS)
    t1 = np.asarray(bn_b, f32) - np.asarray(bn_m, f32) * s1

    qkv_w = np.asarray(qkv_w, f32)
    Wq, Wk, Wv = qkv_w[:C], qkv_w[C:2 * C], qkv_w[2 * C:]
    scale = D ** -0.5
    Wq_f = (Wq * s1[None, :]) * scale
    bq = (Wq @ t1) * scale
    Wk_f = Wk * s1[None, :]
    Wv_f = Wv * s1[None, :]
    bv = Wv @ t1

    proj_w = np.asarray(proj_w, f32)
    Wp_f = ls1v[:, None] * proj_w
    pb = ls1v * (np.asarray(proj_b, f32) + proj_w @ bv)

    fs = np.asarray(fbn_g, f32) / np.sqrt(np.asarray(fbn_v, f32) + EPS)
    ft = np.asarray(fbn_b, f32) - np.asarray(fbn_m, f32) * fs
    dww = np.asarray(dw_w, f32)[:, 0] * fs[:, None, None]      # [C,7,7]
    fc1_w = np.asarray(fc1_w, f32)
    fb1 = np.asarray(fc1_b, f32) + fc1_w @ ft
    fc2_w = np.asarray(fc2_w, f32)
    Wf2 = ls2v[:, None] * fc2_w
    fb2 = ls2v * np.asarray(fc2_b, f32)

    # combined qkv weight, lhsT layout [CCH, 128, 3C]
    Wqkv = np.concatenate([Wq_f, Wk_f, Wv_f], axis=0)          # [3C, C]
    qkvw_t = np.ascontiguousarray(
        Wqkv.T.reshape(CCH, P, 3 * C)).astype(bf)
    projw_t = np.ascontiguousarray(Wp_f.T.reshape(CCH, P, C)).astype(bf)
    fc1w_t = np.ascontiguousarray(fc1_w.T.reshape(CCH, P, HID)).astype(bf)
    fc2w_t = np.ascontiguousarray(Wf2.T.reshape(HCH, P, C)).astype(bf)

    def colmajor(v, nch):
        return np.ascontiguousarray(v.reshape(nch, P).T).astype(f32)

    qb_t = colmajor(bq, CCH)
    pb_t = colmajor(pb, CCH)
    f1b_t = colmajor(fb1, HCH)
    f2b_t = colmajor(fb2, CCH)
    s1_t = colmajor(s1, CCH)
    t1_t = colmajor(t1, CCH)

    # dwtap [P, CCH, 49]
    dwt = np.ascontiguousarray(
        dww.reshape(CCH, P, 49).transpose(1, 0, 2)).astype(f32)
    # dwdiag [NPE, CCH, P, P]
    dwd = np.zeros((len(PE_TAPS), CCH, P, P), f32)
    for i, (dy, dx) in enumerate(PE_TAPS):
        for cc in range(CCH):
            np.fill_diagonal(dwd[i, cc], dww[cc * P:(cc + 1) * P, dy, dx])
    dwd = dwd.astype(bf)

    sel = np.zeros((2, P), f32)
    sel[0, 0:32] = 1.0
    sel[1, 32:64] = 1.0

    # x shards: [CCH, 128, BPC, N]
    xr = x.reshape(NCORES, BPC, C, N)
    shared = dict(qkvw=qkvw_t, qb=qb_t, projw=projw_t, projb=pb_t,
                  fc1w=fc1w_t, fc1b=f1b_t, fc2w=fc2w_t, fc2b=f2b_t,
                  s1d=s1_t, t1d=t1_t, dwtap=dwt, dwdiag=dwd, selw=sel)
    in_maps = []
    for c in range(NCORES):
        xc = np.ascontiguousarray(
            xr[c].reshape(BPC, CCH, P, N).transpose(1, 2, 0, 3))
        in_maps.append(dict(shared, xin=xc))
    return in_maps


def _get_nc():
    if "nc" not in _CACHE:
        _CACHE["nc"] = _build_nc()
    return _CACHE["nc"]


def _gather(results):
    outs = []
    for c in range(NCORES):
        oc = results[c]["out"]              # [CCH, P, BPC, N]
        outs.append(oc.transpose(2, 0, 1, 3).reshape(BPC, C, H, W))
    return np.concatenate(outs, axis=0).astype(np.float32)


def kernel(**inputs):
    nc = _get_nc()
    in_maps = _prep_inputs(**inputs)
    res = run_bass_kernel_spmd(nc, in_maps, list(range(NCORES)))
    return _gather(res.results)


if __name__ == "__main__":
    rng = np.random.default_rng(0)
    ins = dict(
        x=rng.normal(size=(B, C, H, W)).astype(np.float32),
        bn_g=1.0 + 0.1 * rng.normal(size=C).astype(np.float32),
        bn_b=0.1 * rng.normal(size=C).astype(np.float32),
        bn_m=0.1 * rng.normal(size=C).astype(np.float32),
        bn_v=rng.uniform(0.5, 1.5, size=C).astype(np.float32),
        qkv_w=0.02 * rng.normal(size=(3 * C, C)).astype(np.float32),
        proj_w=0.02 * rng.normal(size=(C, C)).astype(np.float32),
        proj_b=np.zeros(C, np.float32),
        dw_w=0.02 * rng.normal(size=(C, 1, 7, 7)).astype(np.float32),
        fbn_g=1.0 + 0.1 * rng.normal(size=C).astype(np.float32),
        fbn_b=0.1 * rng.normal(size=C).astype(np.float32),
        fbn_m=0.1 * rng.normal(size=C).astype(np.float32),
        fbn_v=rng.uniform(0.5, 1.5, size=C).astype(np.float32),
        fc1_w=0.02 * rng.normal(size=(HID, C)).astype(np.float32),
        fc1_b=np.zeros(HID, np.float32),
        fc2_w=0.02 * rng.normal(size=(C, HID)).astype(np.float32),
        fc2_b=np.zeros(C, np.float32),
        ls1=1e-5 * np.ones((C, 1, 1), np.float32),
        ls2=1e-5 * np.ones((C, 1, 1), np.float32),
    )
    o = kernel(**ins)
    print("out", o.shape, o.dtype, float(np.abs(o).max()))

